# revision 10
# baseline (speedup 1.0000x reference)
"""Trainium2 Bass kernel: multi-head attention (B=4, T=2048, D=2048, H=16).

Sharding: 8 cores = 4 batches x 2 head-groups. Each core: one batch, 8 heads
(f-slice of 1024 cols of QKV projections / rows of out-projection). Host sums
the two partial out-projection results per batch and adds the output bias.

v3: single fused pipeline; fp8 only where its noise is affordable:
  - Q/K projections run as fp8e4 DoubleRow matmuls (2x PE throughput, K=256
    per instruction). Host pre-scales x by 16 and Wq/Wk by 4096 into the fp8
    normal range; the DVE epilogue rescales PSUM by 2^-16 and adds the bias,
    emitting q/k in bf16 (re-quantizing them to fp8 would add another 2.6%
    logit noise and push rel err over the 2e-2 gate; measured here 1.66e-2).
  - Scores/P/V/out-projection stay bf16: quantization noise on P or V passes
    unattenuated into the output (the attention average has the same
    cancellation factor as the noise), and K=128 matmuls get no DoubleRow
    speedup anyway.

Schedule (emission order = per-engine execution order):
  P1  QK(h) units head-major from resident x8 (t-block-major DMA so the
      first chain starts ~3us in), with S(0)/S(1) woven in so ScalarE's exp
      pipeline starts ~30us in.  wv preloads into a flat buffer during P1.
  P2  16 V(tc) units (x-bf16 streamed); exp runs ~2 iterations ahead
      (pt pool = 2 slots) then stalls until PV frees slots.
  P3  rounds r: PV(r) + S(r+3) + (r>=8) out-proj C units for t<1024 (ready
      once half-0 PVs finish; iteration order is (half, head)); then the
      remaining PVs and the rest of C.  yT reuses wv's 32KB slot (same pool
      tag); vpool/ytp allocations are deferred so every phase fits in SBUF.
"""

import sys

if "/opt/trn_rl_repo" not in sys.path:
    sys.path.insert(0, "/opt/trn_rl_repo")

import numpy as np
import ml_dtypes

D = 2048          # d_model
T = 2048          # sequence length
B = 4             # batch
H = 16            # total heads
DH = 128          # head dim
GROUPS = 2        # head groups (tensor-parallel factor per batch)
HG = H // GROUPS  # heads per core = 8
F = HG * DH       # per-core projection width = 1024
P = 128
DC = D // P       # 16 contraction chunks
TC = T // P       # 16 t chunks
NCORES = 8
SCALE = float(1.0 / np.sqrt(DH))

XS = 16.0         # host scale on x for fp8
WS = 4096.0       # host scale on Wq/Wk for fp8
EPI = 1.0 / (XS * WS)          # PSUM -> q/k bf16 rescale

_PROGRAM = None


def _build_program():
    import concourse.bass as bass
    import concourse.tile as tile
    from concourse import bacc, mybir
    from concourse.bass import ts, ds
    from concourse.masks import make_identity

    bf16 = mybir.dt.bfloat16
    fp8 = mybir.dt.float8e4
    f32 = mybir.dt.float32
    DR = mybir.MatmulPerfMode.DoubleRow
    Exp = mybir.ActivationFunctionType.Exp
    Mult = mybir.AluOpType.mult
    Add = mybir.AluOpType.add

    nc = bacc.Bacc("TRN2", target_bir_lowering=False, debug=False,
                   num_devices=NCORES)

    x8_d = nc.dram_tensor("x8", [DC, P, T], fp8, kind="ExternalInput")
    xb_d = nc.dram_tensor("xb", [DC, P, T], bf16, kind="ExternalInput")
    wq8_d = nc.dram_tensor("wq8", [DC, P, F], fp8, kind="ExternalInput")
    wk8_d = nc.dram_tensor("wk8", [DC, P, F], fp8, kind="ExternalInput")
    wv_d = nc.dram_tensor("wv", [DC, P, F], bf16, kind="ExternalInput")
    wo_d = nc.dram_tensor("wo", [HG, P, D], bf16, kind="ExternalInput")
    bq_d = nc.dram_tensor("bq", [P, HG], f32, kind="ExternalInput")
    bk_d = nc.dram_tensor("bk", [P, HG], f32, kind="ExternalInput")
    bv_d = nc.dram_tensor("bv", [P, HG], f32, kind="ExternalInput")
    out_d = nc.dram_tensor("out", [DC, P, T], bf16, kind="ExternalOutput")

    with tile.TileContext(nc) as tc:
        from contextlib import ExitStack
        with ExitStack() as ctx:
            # ---- persistent pools (entry order = allocation order) ----
            const = ctx.enter_context(tc.tile_pool(name="const", bufs=1))
            qkt = ctx.enter_context(tc.tile_pool(name="qkt", bufs=1))
            ptp = ctx.enter_context(tc.tile_pool(name="ptp", bufs=2))
            # 32KB flat slot: wv during P1/P2, recycled as yT for P3
            wyf = ctx.enter_context(tc.tile_pool(name="wyf", bufs=1))
            ps_a = ctx.enter_context(
                tc.tile_pool(name="ps_a", bufs=2, space="PSUM"))
            ps_st = ctx.enter_context(
                tc.tile_pool(name="ps_st", bufs=2, space="PSUM"))
            ps_pv = ctx.enter_context(
                tc.tile_pool(name="ps_pv", bufs=1, space="PSUM"))
            ps_tr = ctx.enter_context(
                tc.tile_pool(name="ps_tr", bufs=1, space="PSUM"))

            ident = const.tile([P, P], bf16, tag="ident")
            make_identity(nc, ident)
            zero_b = const.tile([P, 1], f32, tag="zerob")
            nc.vector.memset(zero_b[:], 0.0)
            bq_sb = const.tile([P, HG], f32, tag="bq")
            bk_sb = const.tile([P, HG], f32, tag="bk")
            bv_sb = const.tile([P, HG], f32, tag="bv")
            nc.sync.dma_start(bq_sb[:], bq_d[:, :])
            nc.sync.dma_start(bk_sb[:], bk_d[:, :])
            nc.sync.dma_start(bv_sb[:], bv_d[:, :])

            qT = [qkt.tile([P, T], bf16, tag=f"qT{h}", name=f"qT{h}")
                  for h in range(HG)]
            kT = [qkt.tile([P, T], bf16, tag=f"kT{h}", name=f"kT{h}")
                  for h in range(HG)]

            wv_t = wyf.tile([P, DC * F], bf16, tag="flat", name="wv_flat")

            def wv_load(dc):
                nc.sync.dma_start(wv_t[:, ds(dc * F, F)], wv_d[dc])

            # ---------------- unit generators ----------------
            def qk_unit(h, wqs, x8):
                for wd, bias, dst, kind in ((wq8_d, bq_sb, qT, "q"),
                                            (wk8_d, bk_sb, kT, "k")):
                    wt = wqs.tile([P, DC, DH], fp8, tag=f"w{kind}",
                                  name=f"w{kind}{h}")
                    nc.sync.dma_start(
                        wt[:],
                        wd[:, :, ds(h * DH, DH)].rearrange("c p d -> p c d"))
                    for tb in range(4):
                        ps = ps_a.tile([P, 512], f32, tag="ps512",
                                       name=f"p{kind}{h}_{tb}")
                        for j in range(8):
                            nc.tensor.matmul(
                                ps[:], wt[:, ds(2 * j, 2)],
                                x8[:, ds(2 * j, 2), ds(tb * 512, 512)],
                                start=(j == 0), stop=(j == 7),
                                perf_mode=DR)
                        nc.vector.tensor_scalar(
                            dst[h][:, ds(tb * 512, 512)], ps[:],
                            EPI, bias[:, ds(h, 1)], Mult, Add)

            def s_chunk(i, pt, kc):
                h, q0 = i % HG, (i // HG) * (T // 2)
                st = ps_st.tile([P, 1024], f32, tag="st",
                                name=f"st{i}_{kc}")
                for qc in range(2):
                    nc.tensor.matmul(
                        st[:, ds(qc * 512, 512)],
                        kT[h][:, ds(kc * P, P)],
                        qT[h][:, ds(q0 + qc * 512, 512)],
                        start=True, stop=True)
                nc.scalar.activation(pt[:, kc], st[:], Exp,
                                     bias=zero_b[:, :], scale=SCALE)

            def s_unit(i):
                pt = ptp.tile([P, TC, T // 2], bf16, tag="pt", name=f"pt{i}")
                for kc in range(TC):
                    s_chunk(i, pt, kc)
                return pt

            def pv_unit(i, pt, v_sb, yT, rsp, ysg, filler=None,
                        cfill=None):
                # filler: callable emitting one small PE work item; pulled
                # twice per q-chunk so PE stays busy while DVE normalizes
                # (pv/tr are single-buffered PSUM)
                h, q0 = i % HG, (i // HG) * (T // 2)
                for qs in range(8):
                    pv = ps_pv.tile([P, DH + 1], f32, tag="pv",
                                    name=f"pv{i}_{qs}")
                    for kc in range(TC):
                        nc.tensor.matmul(
                            pv[:], pt[:, kc, ds(qs * P, P)],
                            v_sb[:, kc, h],
                            start=(kc == 0), stop=(kc == TC - 1))
                    rs = rsp.tile([P, 1], f32, tag="rs", name=f"rs{i}_{qs}")
                    nc.vector.reciprocal(rs[:], pv[:, DH:DH + 1])
                    yst = ysg.tile([P, P], bf16, tag="yst",
                                   name=f"yst{i}_{qs}")
                    nc.vector.tensor_scalar_mul(yst[:], pv[:, 0:DH], rs[:])
                    if filler is not None:
                        filler()
                        filler()
                    tr = ps_tr.tile([P, P], bf16, tag="tr",
                                    name=f"tr{i}_{qs}")
                    nc.tensor.transpose(tr[:], yst[:], ident[:])
                    nc.vector.tensor_scalar_add(
                        yT[:, ds((h * T) + q0 + qs * P, P)], tr[:],
                        bv_sb[:, ds(h, 1)])
                    if cfill is not None and qs % 2 == 1:
                        cfill()

            def v_unit(tc_, xbs, v_sb):
                xb = xbs.tile([P, DC, P], bf16, tag="xb", name=f"xb{tc_}")
                nc.sync.dma_start(
                    xb[:],
                    xb_d[:, :, ds(tc_ * P, P)].rearrange("c p t -> p c t"))
                for half in range(2):
                    ps = ps_a.tile([P, 512], f32, tag="ps512",
                                   name=f"pv512_{tc_}_{half}")
                    for dc in range(DC):
                        nc.tensor.matmul(
                            ps[:], xb[:, dc],
                            wv_t[:, ds(dc * F + half * 512, 512)],
                            start=(dc == 0), stop=(dc == DC - 1))
                    nc.vector.tensor_copy(
                        v_sb[:, tc_, ds(half * 4, 4), 0:DH],
                        ps[:].rearrange("p (h d) -> p h d", d=DH))

            def wo_tile(dch, wop):
                wo_t = wop.tile([P, HG, P], bf16, tag="wo", name=f"wo{dch}")
                nc.sync.dma_start(
                    wo_t[:],
                    wo_d[:, :, ds(dch * P, P)].rearrange("h p d -> p h d"))
                return wo_t

            def c_unit(dch, tcb, wo_t, yT, osb):
                pso = ps_a.tile([P, 512], f32, tag="ps512",
                                name=f"pso{dch}_{tcb}")
                for fc in range(HG):
                    nc.tensor.matmul(
                        pso[:], wo_t[:, fc],
                        yT[:, ds(fc * T + tcb * 512, 512)],
                        start=(fc == 0), stop=(fc == HG - 1))
                ot = osb.tile([P, 512], bf16, tag="ot",
                              name=f"ot{dch}_{tcb}")
                nc.vector.tensor_copy(ot[:], pso[:])
                nc.sync.dma_start(out_d[dch][:, ds(tcb * 512, 512)], ot[:])

            # ---------------- emission schedule ----------------
            NIT = 2 * HG  # 16 iterations (half, head)
            LOOK = 2      # pt slots
            pts = {}

            # Phase 1: Q/K projections (x8 resident) + first S units
            with tc.tile_pool(name="x8p", bufs=1) as x8p, \
                 tc.tile_pool(name="wqs", bufs=2) as wqs:
                x8 = x8p.tile([P, DC, T], fp8, tag="x8")
                for tb in range(8):
                    nc.sync.dma_start(
                        x8[:, :, ds(tb * 256, 256)],
                        x8_d[:, :, ds(tb * 256, 256)].rearrange(
                            "c p t -> p c t"))
                qk_unit(0, wqs, x8)
                wv_load(0), wv_load(1)
                qk_unit(1, wqs, x8)
                wv_load(2), wv_load(3)
                pts[0] = s_unit(0)
                qk_unit(2, wqs, x8)
                wv_load(4), wv_load(5)
                pts[1] = s_unit(1)
                qk_unit(3, wqs, x8)
                for h in range(4, HG):
                    wv_load(2 * h - 2), wv_load(2 * h - 1)
                    qk_unit(h, wqs, x8)
                wv_load(14), wv_load(15)

            # Phase 2: V projection (wv already resident; x-bf16 streamed)
            vpool = ctx.enter_context(tc.tile_pool(name="vpool", bufs=1))
            v_sb = vpool.tile([P, TC, HG, DH + 1], bf16, tag="v")
            nc.vector.memset(v_sb[:, :, :, DH:DH + 1], 1.0)
            with tc.tile_pool(name="xbs", bufs=3) as xbs:
                for tc_ in range(TC):
                    v_unit(tc_, xbs, v_sb)

            # Phase 3: rounds (PV + lookahead S) + out-projection
            yT_t = wyf.tile([P, DC * F], bf16, tag="flat", name="yT_flat")
            wop = ctx.enter_context(tc.tile_pool(name="wop", bufs=6))
            ysg = ctx.enter_context(tc.tile_pool(name="ysg", bufs=2))
            rsp = ctx.enter_context(tc.tile_pool(name="rsp", bufs=4))
            osb = ctx.enter_context(tc.tile_pool(name="osb", bufs=2))

            pts[2] = s_unit(2)
            wo_tiles = {0: wo_tile(0, wop), 1: wo_tile(1, wop)}
            # C work for t<1024 is ready after PV7 (half 0 complete)
            c_lo = [(dch, tcb) for dch in range(DC) for tcb in (0, 1)]
            ci = 0

            def emit_c(n):
                nonlocal ci
                for _ in range(n):
                    if ci >= len(c_lo):
                        return
                    dch, tcb = c_lo[ci]
                    if dch not in wo_tiles:
                        wo_tiles[dch] = wo_tile(dch, wop)
                    if dch + 2 < DC and dch + 2 not in wo_tiles:
                        wo_tiles[dch + 2] = wo_tile(dch + 2, wop)
                    c_unit(dch, tcb, wo_tiles[dch], yT_t, osb)
                    ci += 1

            for r in range(NIT):
                i2 = r + LOOK + 1
                if i2 < NIT:
                    pt2 = ptp.tile([P, TC, T // 2], bf16, tag="pt",
                                   name=f"pt{i2}")
                    pts[i2] = pt2
                    kcs = iter(range(TC))

                    def filler(i2=i2, pt2=pt2, kcs=kcs):
                        kc = next(kcs, None)
                        if kc is not None:
                            s_chunk(i2, pt2, kc)
                else:
                    filler = None
                cfill = (lambda: emit_c(1)) if r >= HG else None
                pv_unit(r, pts.pop(r), v_sb, yT_t, rsp, ysg, filler, cfill)
            emit_c(len(c_lo))
            for dch in range(DC):
                # fresh fetch: reusing the round-phase tiles would extend
                # their lifetime past later slot recycles (deadlock)
                wo_t = wo_tile(dch, wop)
                c_unit(dch, 2, wo_t, yT_t, osb)
                c_unit(dch, 3, wo_t, yT_t, osb)

    nc.compile()
    return nc


def _get_program():
    global _PROGRAM
    if _PROGRAM is None:
        _PROGRAM = _build_program()
    return _PROGRAM


def _prep_inputs(x, Wq, bq, Wk, bk, Wv, bv, Wo, bo):
    """Build the 8 per-core input maps (host-side sharding, free)."""
    bf = ml_dtypes.bfloat16
    e4 = ml_dtypes.float8_e4m3
    x = np.asarray(x, dtype=np.float32)
    WqT = np.ascontiguousarray(np.asarray(Wq, np.float32).T)  # [D, D]
    WkT = np.ascontiguousarray(np.asarray(Wk, np.float32).T)
    WvT = np.ascontiguousarray(np.asarray(Wv, np.float32).T)
    WoT = np.ascontiguousarray(np.asarray(Wo, np.float32).T)  # [D, D] (f, d)
    wq8_full = (WS * WqT).astype(e4)
    wk8_full = (WS * WkT).astype(e4)

    xT_by_b = []
    for b in range(B):
        xT = np.ascontiguousarray(x[b].T)
        xT_by_b.append((
            (XS * xT).astype(e4).reshape(DC, P, T),
            xT.astype(bf).reshape(DC, P, T),
        ))

    in_maps = []
    for c in range(NCORES):
        b, g = divmod(c, GROUPS)
        fsl = slice(g * F, (g + 1) * F)
        x8, xb = xT_by_b[b]
        m = {
            "x8": x8,
            "xb": xb,
            "wq8": np.ascontiguousarray(wq8_full[:, fsl]).reshape(DC, P, F),
            "wk8": np.ascontiguousarray(wk8_full[:, fsl]).reshape(DC, P, F),
            "wv": np.ascontiguousarray(WvT[:, fsl]).astype(bf).reshape(
                DC, P, F),
            "wo": np.ascontiguousarray(WoT[fsl, :]).astype(bf).reshape(
                HG, P, D),
            "bq": np.ascontiguousarray(
                np.asarray(bq, np.float32)[fsl].reshape(HG, P).T),
            "bk": np.ascontiguousarray(
                np.asarray(bk, np.float32)[fsl].reshape(HG, P).T),
            "bv": np.ascontiguousarray(
                np.asarray(bv, np.float32)[fsl].reshape(HG, P).T),
        }
        in_maps.append(m)
    return in_maps


def _combine(results, bo):
    bo = np.asarray(bo, np.float32)
    out = np.empty((B, T, D), dtype=np.float32)
    for b in range(B):
        oT = (results[b * GROUPS]["out"].reshape(D, T).astype(np.float32)
              + results[b * GROUPS + 1]["out"].reshape(D, T).astype(
                  np.float32))
        out[b] = oT.T + bo[None, :]
    return out


def kernel(x, Wq, bq, Wk, bk, Wv, bv, Wo, bo):
    from concourse.bass_utils import run_bass_kernel_spmd

    nc = _get_program()
    in_maps = _prep_inputs(x, Wq, bq, Wk, bk, Wv, bv, Wo, bo)
    res = run_bass_kernel_spmd(nc, in_maps, list(range(NCORES))).results
    return _combine(res, bo)
